# revision 33
# baseline (speedup 1.0000x reference)
"""Trainium2 Bass kernel for nn_AttentionBlock (sparse_attention).

Full-input contract: kernel(**inputs) takes the complete tensors and returns
the complete [4, 512, 512] output. Internally shards over 8 NeuronCores as
(batch, i-half): core c handles batch c//2, query rows (c%2)*256 ..+256.

Key layout choice: refCov is transposed host-side into pair-packed
[NPAIR, 2*53, L] bf16 so the device L1 matmul consumes it directly with the
feature dim on partitions (no on-device repack/transpose). Masking is
additive (-1e30 on invalid j columns, folded into the L2 evacuation and a
rank-1 PSUM init for scores); rows with an invalid query index are fixed at
the end with a host-precomputed uniform-attention row.

Self-contained: hardcodes all shapes; no sibling imports.
"""

import sys

if "/opt/trn_rl_repo" not in sys.path:
    sys.path.insert(0, "/opt/trn_rl_repo")

from contextlib import ExitStack

import numpy as np
import ml_dtypes

import concourse.bass as bass
import concourse.tile as tile
from concourse import bacc, mybir
from concourse.bass_utils import run_bass_kernel_spmd
from concourse.masks import make_identity

F32 = mybir.dt.float32
BF16 = mybir.dt.bfloat16
U8 = mybir.dt.uint8
U32 = mybir.dt.uint32
AF = mybir.ActivationFunctionType
ALU = mybir.AluOpType
AX = mybir.AxisListType

NPBF16 = ml_dtypes.bfloat16

NEG = -1.0e30

# Full-problem constants
B, L_FULL, H, NH = 4, 512, 512, 8
DK = H // NH  # 64
CIN, CHID = 53, 32  # repr MLP dims
N_CORES = 8


def build_program(L, LI, has_bq, has_bk, has_bv, has_bo, has_r2b, r2b_vals,
                  trace_sim=False):
    """One-core program: attention block over LI query rows, L context.

    L multiple of 128; LI multiple of 16 (and of 128 when >= 128).
    r2b_vals: python floats baked as immediates when has_r2b.
    """
    assert L % 128 == 0 and LI % 16 == 0
    NJB = L // 128            # j blocks (k-chunks of v / wT)
    NPAIR = LI // 2           # i-row pairs (layer-1 packing)
    IBS = min(128, LI)        # i-block size for attention tiles
    NIB = LI // IBS           # i-blocks
    NHC = H // 128            # h chunks (4)
    assert (IBS // 2) % 8 == 0
    NCHUNK = IBS // 16        # 16-i-row chunks per i-block (4 groups of 2 pairs)
    scale = float(1.0 / np.sqrt(DK))

    nc = bacc.Bacc()

    xfull = nc.dram_tensor("xfull", [L, H], F32, kind="ExternalInput")
    xq_d = nc.dram_tensor("xq", [LI, H], F32, kind="ExternalInput")
    refc_d = nc.dram_tensor("refc", [NPAIR // 4, 128, 4, L], BF16, kind="ExternalInput")
    masku8 = nc.dram_tensor("masku8", [L, 1], U8, kind="ExternalInput")
    ioff_d = nc.dram_tensor("ioff", [LI, 1], U8, kind="ExternalInput")
    wq_d = nc.dram_tensor("wq", [H, H], BF16, kind="ExternalInput")
    wk_d = nc.dram_tensor("wk", [H, H], BF16, kind="ExternalInput")
    wv_d = nc.dram_tensor("wv", [H, H], BF16, kind="ExternalInput")
    wo_d = nc.dram_tensor("wo", [H, H], BF16, kind="ExternalInput")
    bq_d = nc.dram_tensor("bq", [H, 1], F32, kind="ExternalInput")
    bk_d = nc.dram_tensor("bk", [H, 1], F32, kind="ExternalInput")
    bv_d = nc.dram_tensor("bv", [H, 1], F32, kind="ExternalInput")
    bo_d = nc.dram_tensor("bo", [H, 1], F32, kind="ExternalInput")
    bd1_d = nc.dram_tensor("bd1h", [2 * CIN, 2 * CHID], BF16, kind="ExternalInput")
    bd2_d = nc.dram_tensor("bd2h", [128, 4 * NH], BF16, kind="ExternalInput")
    r1b_d = nc.dram_tensor("r1b", [CHID, 1], F32, kind="ExternalInput")
    fixr_d = nc.dram_tensor("fixrow", [H, 1], F32, kind="ExternalInput")
    negpre_d = nc.dram_tensor("negpre", [NH * L, 1], BF16, kind="ExternalInput")
    lng_d = nc.dram_tensor("lng", [H, 1], F32, kind="ExternalInput")
    lnb_d = nc.dram_tensor("lnb", [H, 1], F32, kind="ExternalInput")
    y_d = nc.dram_tensor("y", [LI, H], F32, kind="ExternalOutput")

    with tile.TileContext(nc, trace_sim=trace_sim) as tc, ExitStack() as ctx:
        P = ctx.enter_context(tc.tile_pool(name="persist", bufs=1))
        pstage = ctx.enter_context(tc.tile_pool(name="stage", bufs=6))
        phid = ctx.enter_context(tc.tile_pool(name="hid", bufs=6))
        pl2s = ctx.enter_context(tc.tile_pool(name="l2s", bufs=3))
        pea = ctx.enter_context(tc.tile_pool(name="ea", bufs=9))
        per = ctx.enter_context(tc.tile_pool(name="er", bufs=4))
        pwts = ctx.enter_context(tc.tile_pool(name="wts", bufs=4))
        pry = ctx.enter_context(tc.tile_pool(name="ry", bufs=2))
        psc = ctx.enter_context(tc.tile_pool(name="sc", bufs=40))
        # PSUM pools — 1+2+1+2+2 = 8 banks
        pp_main = ctx.enter_context(tc.tile_pool(name="ppmain", bufs=1, space="PSUM"))
        pp_l1 = ctx.enter_context(tc.tile_pool(name="ppl1", bufs=2, space="PSUM"))
        pp_l2 = ctx.enter_context(tc.tile_pool(name="ppl2", bufs=2, space="PSUM"))
        pp_sp = ctx.enter_context(tc.tile_pool(name="ppsp", bufs=2, space="PSUM"))
        pp_wav = ctx.enter_context(tc.tile_pool(name="ppwav", bufs=1, space="PSUM"))

        # ---------- constants / weights ----------
        ident = P.tile([128, 128], F32, tag="ident")
        make_identity(nc, ident)
        identB = P.tile([128, 128], BF16, tag="identB")
        make_identity(nc, identB)

        # prologue loads go on the gpsimd (SWDGE) queue so the sync HWDGE ring
        # starts streaming refc groups immediately
        bd1 = P.tile([2 * CIN, 2 * CHID], BF16, tag="bd1")  # [106, 64]
        nc.sync.dma_start(out=bd1, in_=bd1_d[:, :])
        bd2 = P.tile([128, 4 * NH], BF16, tag="bd2")  # [128, 32]
        nc.sync.dma_start(out=bd2, in_=bd2_d[:, :])

        r1b4 = P.tile([128, 1], F32, tag="r1b4")
        for g in range(4):
            nc.sync.dma_start(out=r1b4[32 * g : 32 * g + CHID, :], in_=r1b_d[:, :])

        w_sb = {}
        for nm, d in (("wq", wq_d), ("wk", wk_d), ("wv", wv_d), ("wo", wo_d)):
            for kk in range(NHC):
                t = P.tile([128, H], BF16, tag=f"{nm}{kk}")
                nc.gpsimd.dma_start(out=t, in_=d[128 * kk : 128 * (kk + 1), :])
                w_sb[(nm, kk)] = t

        bias_sb = {}
        for nm, d, has in (("bq", bq_d, has_bq), ("bk", bk_d, has_bk), ("bv", bv_d, has_bv)):
            if has:
                for kk in range(NHC):
                    t = P.tile([128, 1], F32, tag=f"{nm}{kk}")
                    nc.gpsimd.dma_start(out=t, in_=d[128 * kk : 128 * (kk + 1), :])
                    bias_sb[(nm, kk)] = t

        # ---------- mask tiles (natural j order) ----------
        mjb_u8 = P.tile([128, L], U8, tag="mjbu8")
        nc.gpsimd.dma_start(
            out=mjb_u8, in_=bass.AP(tensor=masku8, offset=0, ap=[[0, 128], [1, L]])
        )
        mjb = P.tile([128, L], F32, tag="mjb")
        nc.vector.tensor_copy(out=mjb, in_=mjb_u8)
        # negadd[p, j] = NEG where mask[j]==0 else 0   (same for all partitions)
        negadd = P.tile([128, L], F32, tag="negadd")
        nc.vector.tensor_scalar(
            out=negadd, in0=mjb, scalar1=0.0, scalar2=NEG,
            op0=ALU.is_equal, op1=ALU.mult,
        )
        ones2 = P.tile([128, 128], BF16, tag="ones2")
        nc.gpsimd.memset(ones2, 1.0)
        negj1 = P.tile([1, L], BF16, tag="negj1")
        nc.vector.tensor_copy(out=negj1, in_=negadd[0:1, :])
        negjB = P.tile([128, L], BF16, tag="negjB")
        nc.vector.tensor_copy(out=negjB, in_=negadd)
        ones_f = P.tile([128, H], F32, tag="ones_f")
        nc.gpsimd.memset(ones_f, 1.0)
        eps_t = P.tile([128, 1], F32, tag="eps_t")
        nc.gpsimd.memset(eps_t, 1e-5)

        # per-block invalid-query masks (for the final row fix)
        ivq = []
        for ib in range(NIB):
            miu = P.tile([IBS, 1], U8, tag=f"miu{ib}")
            nc.gpsimd.dma_start(out=miu, in_=ioff_d[IBS * ib : IBS * (ib + 1), :])
            mif = P.tile([IBS, 1], F32, tag=f"mif{ib}")
            nc.vector.tensor_copy(out=mif, in_=miu)
            iv = P.tile([IBS, H], U32, tag=f"ivq{ib}")
            nc.vector.tensor_scalar(
                out=iv, in0=ones_f[0:IBS, :], scalar1=mif, scalar2=0.0,
                op0=ALU.mult, op1=ALU.is_equal,
            )
            ivq.append(iv)

        g_bc = P.tile([128, H], F32, tag="g_bc")
        nc.gpsimd.dma_start(
            out=g_bc, in_=bass.AP(tensor=lng_d, offset=0, ap=[[0, 128], [1, H]])
        )
        b_bc = P.tile([128, H], F32, tag="b_bc")
        nc.gpsimd.dma_start(
            out=b_bc, in_=bass.AP(tensor=lnb_d, offset=0, ap=[[0, 128], [1, H]])
        )
        mvb = P.tile([128, H], F32, tag="mvb")
        nc.gpsimd.dma_start(
            out=mvb, in_=bass.AP(tensor=fixr_d, offset=0, ap=[[0, 128], [1, H]])
        )
        bo_bc = None
        if has_bo:
            bo_bc = P.tile([128, H], F32, tag="bo_bc")
            nc.gpsimd.dma_start(
                out=bo_bc, in_=bass.AP(tensor=bo_d, offset=0, ap=[[0, 128], [1, H]])
            )

        # ---------- x loads, transposes (natural j chunks) ----------
        xf = []
        for t in range(NJB):
            xt = P.tile([128, H], F32, tag=f"xf{t}")
            nc.gpsimd.dma_start(out=xt, in_=xfull[128 * t : 128 * (t + 1), :])
            xf.append(xt)
        xq_sb = []
        for ib in range(NIB):
            xt = P.tile([IBS, H], F32, tag=f"xq{ib}")
            nc.gpsimd.dma_start(out=xt, in_=xq_d[IBS * ib : IBS * (ib + 1), :])
            xq_sb.append(xt)

        evac_rot = [0]

        def evac_copy(out, in_):
            # gpsimd cannot access PSUM on TRN2: rotate scalar/vector only
            r = evac_rot[0] % 2
            evac_rot[0] += 1
            if r == 0:
                nc.scalar.copy(out=out, in_=in_)
            else:
                nc.vector.tensor_copy(out=out, in_=in_)

        xT = []  # [h-chunk][128, L] bf16 — transposed full x (for k, v)
        for hc in range(NHC):
            ps = pp_main.tile([128, L], F32, tag="big")
            for jt in range(NJB):
                nc.tensor.transpose(
                    out=ps[:, 128 * jt : 128 * (jt + 1)],
                    in_=xf[jt][:, 128 * hc : 128 * (hc + 1)],
                    identity=ident,
                )
            xs = P.tile([128, L], BF16, tag=f"xT{hc}")
            evac_copy(xs, ps)
            xT.append(xs)
        xqT = []  # [h-chunk][128, LI] bf16 — transposed xq (for q)
        for hc in range(NHC):
            ps = pp_main.tile([128, LI], F32, tag="big")
            for ib in range(NIB):
                nc.tensor.transpose(
                    out=ps[:, IBS * ib : IBS * (ib + 1)],
                    in_=xq_sb[ib][:, 128 * hc : 128 * (hc + 1)],
                    identity=ident[0:IBS, 0:IBS],
                )
            xs = P.tile([128, LI], BF16, tag=f"xqT{hc}")
            evac_copy(xs, ps)
            xqT.append(xs)

        # ---------- q/k/v projections (bf16) ----------
        qT = []
        for t in range(NHC):
            ps = pp_main.tile([128, LI], F32, tag="big")
            for kk in range(NHC):
                nc.tensor.matmul(
                    out=ps, lhsT=w_sb[("wq", kk)][:, 128 * t : 128 * (t + 1)],
                    rhs=xqT[kk], start=(kk == 0), stop=(kk == NHC - 1),
                )
            s = P.tile([128, LI], BF16, tag=f"qT{t}")
            if has_bq:
                nc.scalar.activation(out=s, in_=ps, func=AF.Identity, bias=bias_sb[("bq", t)])
            else:
                evac_copy(s, ps)
            qT.append(s)
        kT = []
        for t in range(NHC):
            ps = pp_main.tile([128, L], F32, tag="big")
            for kk in range(NHC):
                nc.tensor.matmul(
                    out=ps, lhsT=w_sb[("wk", kk)][:, 128 * t : 128 * (t + 1)],
                    rhs=xT[kk], start=(kk == 0), stop=(kk == NHC - 1),
                )
            s = P.tile([128, L], BF16, tag=f"kT{t}")
            if has_bk:
                nc.scalar.activation(out=s, in_=ps, func=AF.Identity, bias=bias_sb[("bk", t)])
            else:
                evac_copy(s, ps)
            kT.append(s)
        v_sb = []
        for t in range(NJB):
            ps = pp_main.tile([128, H], F32, tag="big")
            for kk in range(NHC):
                nc.tensor.matmul(
                    out=ps, lhsT=xT[kk][:, 128 * t : 128 * (t + 1)],
                    rhs=w_sb[("wv", kk)],
                    start=(kk == 0), stop=(kk == NHC - 1),
                )
            s = P.tile([128, H], BF16, tag=f"v{t}")
            evac_copy(s, ps)  # bv folded into avT evac
            v_sb.append(s)

        # persistent per-head exp-numerator holders for the ref path
        refS = [
            P.tile([IBS, NH, L], BF16, tag=f"refS{ib}", name=f"refS{ib}")
            for ib in range(NIB)
        ]
        aoT = [P.tile([128, LI], BF16, tag=f"aoT{t}", name=f"aoT{t}") for t in range(NHC)]

        # ---------- main per-i-block phases ----------
        for ib in range(NIB):
            # ---- repr-MLP over this i-block's rows ----
            for c in range(NCHUNK):
                # two 128-partition DMAs per 8-pair chunk (rows 106:128 are
                # host zero-pad): the runtime spreads a full-partition dynamic
                # DMA across all 16 SDMA engines
                stgs = []
                for hh in range(2):
                    stg = pstage.tile([128, 4, L], BF16, tag="stg")
                    nc.sync.dma_start(
                        out=stg, in_=refc_d[(ib * NCHUNK + c) * 2 + hh]
                    )
                    stgs.append(stg)
                hids = []
                for gg in range(4):  # 4 groups of 2 pairs each
                    stg = stgs[gg // 2]
                    l1p = pp_l1.tile([128, L], F32, tag="l1")
                    for z in range(2):
                        nc.tensor.matmul(
                            out=l1p[64 * z : 64 * z + 64, :],
                            lhsT=bd1, rhs=stg[0 : 2 * CIN, 2 * (gg % 2) + z, :],
                            start=True, stop=True, tile_position=(0, 64 * z),
                        )
                    hid = phid.tile([128, L], BF16, tag="hid")
                    if gg % 2 == 0:
                        nc.scalar.activation(
                            out=hid, in_=l1p, func=AF.Relu, bias=r1b4,
                        )
                    else:
                        nc.vector.tensor_scalar(
                            out=hid, in0=l1p, scalar1=r1b4, scalar2=0.0,
                            op0=ALU.add, op1=ALU.max,
                        )
                    hids.append(hid)
                l2p = pp_l2.tile([128, L], F32, tag="l2")
                for q in range(4):
                    nc.tensor.matmul(
                        out=l2p[32 * q : 32 * q + 32, :],
                        lhsT=bd2, rhs=hids[q], start=True, stop=False,
                        tile_position=(0, 32 * q), skip_group_check=True,
                    )
                # accumulate the NEG j-mask with a rank-1 matmul so the evac
                # below is a plain (gap-free) scalar copy
                nc.tensor.matmul(
                    out=l2p, lhsT=ones2[0:1, :], rhs=negj1,
                    start=False, stop=True, skip_group_check=True,
                )
                l2s = pl2s.tile([128, L], BF16, tag="l2s")
                nc.scalar.copy(out=l2s, in_=l2p)
                # de-interleave rows -> refS[i, head, :]: l2s partition
                # p = 32q + 8ii + nh pairs 1:1, in natural order, with the
                # (i, nh) runs of refS[16c:16c+16] — a single DMA suffices
                eng = nc.sync if c % 2 == 0 else nc.gpsimd
                eng.dma_start(
                    out=refS[ib][16 * c : 16 * c + 16, :, :],
                    in_=l2s,
                )

            # ---- attention + ref softmax + combine + AV per head ----
            for nh in range(NH):
                t, s = nh // 2, nh % 2
                sp = pp_sp.tile([IBS, L], F32, tag="sp")
                # psum init: 64*NEG on invalid j columns, issued at this
                # head's PE row group so adjacent heads run concurrently
                nc.tensor.matmul(
                    out=sp, lhsT=ones2[64 * s : 64 * s + 64, 0:IBS],
                    rhs=negjB[64 * s : 64 * s + 64, :],
                    start=True, stop=False, skip_group_check=True,
                )
                nc.tensor.matmul(
                    out=sp,
                    lhsT=qT[t][64 * s : 64 * s + 64, IBS * ib : IBS * (ib + 1)],
                    rhs=kT[t][64 * s : 64 * s + 64, :],
                    start=False, stop=True, skip_group_check=True,
                )
                ea_t = pea.tile([IBS, L], BF16, tag="ea")
                sa = psc.tile([IBS, 1], F32, tag="sa")
                nc.scalar.activation(
                    out=ea_t, in_=sp, func=AF.Exp, scale=scale, accum_out=sa
                )

                rt = refS[ib][:, nh, :]
                if has_r2b:
                    nc.vector.tensor_scalar_add(out=rt, in0=rt, scalar1=float(r2b_vals[nh]))
                er_t = per.tile([IBS, L], BF16, tag="er")
                sr = psc.tile([IBS, 1], F32, tag="sr")
                nc.scalar.activation(out=er_t, in_=rt, func=AF.Exp, accum_out=sr)

                isa = psc.tile([IBS, 1], F32, tag="isa")
                nc.vector.reciprocal(out=isa, in_=sa)
                isr = psc.tile([IBS, 1], F32, tag="isr")
                nc.vector.reciprocal(out=isr, in_=sr)
                # w = ea/sa + er/sr (0.5 factor folded into avT evac scale)
                nc.scalar.activation(out=ea_t, in_=ea_t, func=AF.Identity, scale=isa)
                nc.vector.scalar_tensor_tensor(
                    out=ea_t, in0=er_t, scalar=isr, in1=ea_t, op0=ALU.mult, op1=ALU.add
                )

                wtp = pp_wav.tile([128, NJB * IBS], BF16, tag="wav")
                for k in range(NJB):
                    nc.tensor.transpose(
                        out=wtp[:, IBS * k : IBS * (k + 1)],
                        in_=ea_t[:, 128 * k : 128 * (k + 1)],
                        identity=identB[0:IBS, 0:IBS],
                    )
                wts = pwts.tile([128, NJB * IBS], BF16, tag="wts")
                nc.vector.tensor_copy(out=wts, in_=wtp)

                avp = pp_wav.tile([64, IBS], F32, tag="wav")
                for k in range(NJB):
                    nc.tensor.matmul(
                        out=avp,
                        lhsT=v_sb[k][:, 64 * nh : 64 * nh + 64],
                        rhs=wts[:, IBS * k : IBS * (k + 1)],
                        start=(k == 0), stop=(k == NJB - 1),
                    )
                if has_bv:
                    nc.scalar.activation(
                        out=aoT[t][64 * s : 64 * s + 64, IBS * ib : IBS * (ib + 1)],
                        in_=avp, func=AF.Identity, scale=0.5,
                        bias=bias_sb[("bv", t)][64 * s : 64 * s + 64, :],
                    )
                else:
                    nc.scalar.activation(
                        out=aoT[t][64 * s : 64 * s + 64, IBS * ib : IBS * (ib + 1)],
                        in_=avp, func=AF.Copy, bias=0.0, scale=0.5,
                    )

            # ---- output projection + residual + row fix + layernorm ----
            pp = pp_main.tile([IBS, H], F32, tag="big")
            for kk in range(NHC):
                nc.tensor.matmul(
                    out=pp, lhsT=aoT[kk][:, IBS * ib : IBS * (ib + 1)],
                    rhs=w_sb[("wo", kk)],
                    start=(kk == 0), stop=(kk == NHC - 1),
                )
            r_t = pry.tile([IBS, H], F32, tag="rt")
            nc.vector.scalar_tensor_tensor(
                out=r_t, in0=pp, scalar=1.0, in1=xq_sb[ib], op0=ALU.mult, op1=ALU.add
            )
            # invalid query rows get the exact uniform-attention result
            fr = pry.tile([IBS, H], F32, tag="fr")
            nc.vector.tensor_add(out=fr, in0=xq_sb[ib], in1=mvb[0:IBS, :])
            nc.vector.copy_predicated(out=r_t, mask=ivq[ib], data=fr)
            if has_bo:
                nc.vector.tensor_add(out=r_t, in0=r_t, in1=bo_bc[0:IBS, :])
            stats = psc.tile([IBS, 6], F32, tag="stats")
            nc.vector.bn_stats(out=stats, in_=r_t)
            mv = psc.tile([IBS, 2], F32, tag="mv")
            nc.vector.bn_aggr(out=mv, in_=stats)
            stdv = psc.tile([IBS, 1], F32, tag="stdv")
            nc.scalar.activation(out=stdv, in_=mv[:, 1:2], func=AF.Sqrt, bias=eps_t[0:IBS, :])
            rstd = psc.tile([IBS, 1], F32, tag="rstd")
            nc.vector.reciprocal(out=rstd, in_=stdv)
            nc.vector.tensor_scalar(
                out=r_t, in0=r_t, scalar1=mv[:, 0:1], scalar2=rstd,
                op0=ALU.subtract, op1=ALU.mult,
            )
            y_t = pry.tile([IBS, H], F32, tag="yt")
            nc.vector.tensor_mul(out=y_t, in0=r_t, in1=g_bc[0:IBS, :])
            nc.vector.tensor_add(out=y_t, in0=y_t, in1=b_bc[0:IBS, :])
            nc.scalar.dma_start(out=y_d[IBS * ib : IBS * (ib + 1), :], in_=y_t)

    nc.compile()  # Bacc legalization: ≤1 sync wait per instruction, etc.
    return nc


def _make_bd1(r1w):
    bd1 = np.zeros((2 * CIN, 2 * CHID), np.float32)
    bd1[0:CIN, 0:CHID] = r1w
    bd1[CIN : 2 * CIN, CHID : 2 * CHID] = r1w
    return bd1.astype(NPBF16)


def _make_bd2(r2w):
    bd2 = np.zeros((128, 4 * NH), np.float32)
    for g in range(4):
        bd2[32 * g : 32 * g + CHID, NH * g : NH * (g + 1)] = r2w
    return bd2.astype(NPBF16)


def _pack_refc(refc_core, maski):
    """[LI, L, CIN] f32 + [LI] bool -> pair-packed transposed [LI//2, 2*CIN, L] bf16.

    Invalid query rows are zeroed so their (masked) softmax stays finite and
    row-constant; those output rows are overwritten by the uniform fix anyway.
    """
    LIc, Lc, _ = refc_core.shape
    rc = np.asarray(refc_core, np.float32) * np.asarray(maski, np.float32)[:, None, None]
    rc = rc.astype(NPBF16)
    rc = rc.reshape(LIc // 2, 2, Lc, CIN).transpose(0, 1, 3, 2)
    rc = rc.reshape(LIc // 2, 2 * CIN, Lc)
    # 4-pair chunks: [C, 128, 4, L] so each load is one contiguous
    # 4KB-per-partition descriptor chain over all 128 partitions (the runtime
    # only engages all 16 SDMA engines for full-partition transfers)
    rc = rc.reshape(LIc // 8, 4, 2 * CIN, Lc).transpose(0, 2, 1, 3)
    out = np.zeros((LIc // 8, 128, 4, Lc), NPBF16)
    out[:, 0 : 2 * CIN] = rc
    return out


_PROG_CACHE = {}


def _get_program(L, LI, flags, r2b_vals):
    key = (L, LI, flags)
    if key not in _PROG_CACHE:
        _PROG_CACHE[key] = build_program(L, LI, *flags, r2b_vals)
    return _PROG_CACHE[key]


def make_in_maps(x, mask, refCov, wq, bq, wk, bk, wv, bv, wo, bo,
                 r1w, r1b, r2w, r2b, ln_g, ln_b, n_cores=N_CORES, LI=None):
    Bc, L, Hc = x.shape
    if LI is None:
        LI = (Bc * L) // n_cores
    f = np.float32
    shared = {
        "wq": np.asarray(wq, f).astype(NPBF16),
        "wk": np.asarray(wk, f).astype(NPBF16),
        "wv": np.asarray(wv, f).astype(NPBF16),
        "wo": np.asarray(wo, f).astype(NPBF16),
        "bq": np.ascontiguousarray(bq, f).reshape(Hc, 1),
        "bk": np.ascontiguousarray(bk, f).reshape(Hc, 1),
        "bv": np.ascontiguousarray(bv, f).reshape(Hc, 1),
        "bo": np.ascontiguousarray(bo, f).reshape(Hc, 1),
        "bd1h": _make_bd1(np.asarray(r1w, f)),
        "bd2h": _make_bd2(np.asarray(r2w, f)),
        "r1b": np.ascontiguousarray(r1b, f).reshape(CHID, 1),
        "lng": np.ascontiguousarray(ln_g, f).reshape(Hc, 1),
        "lnb": np.ascontiguousarray(ln_b, f).reshape(Hc, 1),
    }
    per_batch = L // LI  # cores per batch
    in_maps = []
    for c in range(n_cores):
        b, half = c // per_batch, c % per_batch
        i0 = half * LI
        m = dict(shared)
        m["xfull"] = np.ascontiguousarray(x[b], f)
        m["xq"] = np.ascontiguousarray(x[b, i0 : i0 + LI], f)
        m["refc"] = _pack_refc(refCov[b, i0 : i0 + LI], mask[b, i0 : i0 + LI])
        m["masku8"] = np.ascontiguousarray(mask[b].astype(np.uint8).reshape(L, 1))
        m["ioff"] = np.ascontiguousarray(
            mask[b, i0 : i0 + LI].astype(np.uint8).reshape(LI, 1)
        )
        # uniform-attention output row for fully-masked queries:
        # mean_j(v) @ wo  (bias bo is added on-device for all rows)
        mean_v = np.asarray(x[b], f).mean(axis=0) @ np.asarray(wv, f) + np.asarray(bv, f)
        m["fixrow"] = np.ascontiguousarray((mean_v @ np.asarray(wo, f)).reshape(Hc, 1), f)
        negrow = np.where(mask[b], 0.0, -1.0e30).astype(NPBF16)
        m["negpre"] = np.ascontiguousarray(np.tile(negrow, NH).reshape(NH * L, 1))
        in_maps.append(m)
    return in_maps, per_batch, LI


def kernel(x, mask, refCov, wq, bq, wk, bk, wv, bv, wo, bo,
           r1w, r1b, r2w, r2b, ln_g, ln_b, trace=False):
    x = np.asarray(x)
    Bc, L, Hc = x.shape
    flags = (
        bool(np.any(bq)), bool(np.any(bk)), bool(np.any(bv)), bool(np.any(bo)),
        bool(np.any(r2b)),
    )
    in_maps, per_batch, LI = make_in_maps(
        x, mask, refCov, wq, bq, wk, bk, wv, bv, wo, bo,
        r1w, r1b, r2w, r2b, ln_g, ln_b,
    )
    nc = _get_program(L, LI, flags, [float(v) for v in np.asarray(r2b).ravel()])
    res = run_bass_kernel_spmd(nc, in_maps, core_ids=list(range(N_CORES)), trace=trace)
    out = np.empty((Bc, L, Hc), np.float32)
    for c in range(N_CORES):
        b, half = c // per_batch, c % per_batch
        out[b, half * LI : (half + 1) * LI] = res.results[c]["y"]
    if trace:
        return out, res
    return out


# revision 34
# speedup vs baseline: 1.0055x; 1.0055x over previous
"""Trainium2 Bass kernel for nn_AttentionBlock (sparse_attention).

Full-input contract: kernel(**inputs) takes the complete tensors and returns
the complete [4, 512, 512] output. Internally shards over 8 NeuronCores as
(batch, i-half): core c handles batch c//2, query rows (c%2)*256 ..+256.

Key layout choice: refCov is transposed host-side into pair-packed
[NPAIR, 2*53, L] bf16 so the device L1 matmul consumes it directly with the
feature dim on partitions (no on-device repack/transpose). Masking is
additive (-1e30 on invalid j columns, folded into the L2 evacuation and a
rank-1 PSUM init for scores); rows with an invalid query index are fixed at
the end with a host-precomputed uniform-attention row.

Self-contained: hardcodes all shapes; no sibling imports.
"""

import sys

if "/opt/trn_rl_repo" not in sys.path:
    sys.path.insert(0, "/opt/trn_rl_repo")

from contextlib import ExitStack

import numpy as np
import ml_dtypes

import concourse.bass as bass
import concourse.tile as tile
from concourse import bacc, mybir
from concourse.bass_utils import run_bass_kernel_spmd
from concourse.masks import make_identity

F32 = mybir.dt.float32
BF16 = mybir.dt.bfloat16
U8 = mybir.dt.uint8
U32 = mybir.dt.uint32
AF = mybir.ActivationFunctionType
ALU = mybir.AluOpType
AX = mybir.AxisListType

NPBF16 = ml_dtypes.bfloat16

NEG = -1.0e30

# Full-problem constants
B, L_FULL, H, NH = 4, 512, 512, 8
DK = H // NH  # 64
CIN, CHID = 53, 32  # repr MLP dims
N_CORES = 8


def build_program(L, LI, has_bq, has_bk, has_bv, has_bo, has_r2b, r2b_vals,
                  trace_sim=False):
    """One-core program: attention block over LI query rows, L context.

    L multiple of 128; LI multiple of 16 (and of 128 when >= 128).
    r2b_vals: python floats baked as immediates when has_r2b.
    """
    assert L % 128 == 0 and LI % 16 == 0
    NJB = L // 128            # j blocks (k-chunks of v / wT)
    NPAIR = LI // 2           # i-row pairs (layer-1 packing)
    IBS = min(128, LI)        # i-block size for attention tiles
    NIB = LI // IBS           # i-blocks
    NHC = H // 128            # h chunks (4)
    assert (IBS // 2) % 8 == 0
    NCHUNK = IBS // 16        # 16-i-row chunks per i-block (4 groups of 2 pairs)
    scale = float(1.0 / np.sqrt(DK))

    nc = bacc.Bacc()

    xfull = nc.dram_tensor("xfull", [L, H], F32, kind="ExternalInput")
    xq_d = nc.dram_tensor("xq", [LI, H], F32, kind="ExternalInput")
    refc_d = nc.dram_tensor("refc", [NPAIR // 4, 128, 4, L], BF16, kind="ExternalInput")
    masku8 = nc.dram_tensor("masku8", [L, 1], U8, kind="ExternalInput")
    ioff_d = nc.dram_tensor("ioff", [LI, 1], U8, kind="ExternalInput")
    wq_d = nc.dram_tensor("wq", [H, H], BF16, kind="ExternalInput")
    wk_d = nc.dram_tensor("wk", [H, H], BF16, kind="ExternalInput")
    wv_d = nc.dram_tensor("wv", [H, H], BF16, kind="ExternalInput")
    wo_d = nc.dram_tensor("wo", [H, H], BF16, kind="ExternalInput")
    bq_d = nc.dram_tensor("bq", [H, 1], F32, kind="ExternalInput")
    bk_d = nc.dram_tensor("bk", [H, 1], F32, kind="ExternalInput")
    bv_d = nc.dram_tensor("bv", [H, 1], F32, kind="ExternalInput")
    bo_d = nc.dram_tensor("bo", [H, 1], F32, kind="ExternalInput")
    bd1_d = nc.dram_tensor("bd1h", [2 * CIN, 2 * CHID], BF16, kind="ExternalInput")
    bd2_d = nc.dram_tensor("bd2h", [128, 4 * NH], BF16, kind="ExternalInput")
    r1b_d = nc.dram_tensor("r1b", [CHID, 1], F32, kind="ExternalInput")
    fixr_d = nc.dram_tensor("fixrow", [H, 1], F32, kind="ExternalInput")
    negpre_d = nc.dram_tensor("negpre", [NH * L, 1], BF16, kind="ExternalInput")
    lng_d = nc.dram_tensor("lng", [H, 1], F32, kind="ExternalInput")
    lnb_d = nc.dram_tensor("lnb", [H, 1], F32, kind="ExternalInput")
    y_d = nc.dram_tensor("y", [LI, H], F32, kind="ExternalOutput")

    with tile.TileContext(nc, trace_sim=trace_sim) as tc, ExitStack() as ctx:
        P = ctx.enter_context(tc.tile_pool(name="persist", bufs=1))
        pstage = ctx.enter_context(tc.tile_pool(name="stage", bufs=6))
        phid = ctx.enter_context(tc.tile_pool(name="hid", bufs=6))
        pl2s = ctx.enter_context(tc.tile_pool(name="l2s", bufs=3))
        pea = ctx.enter_context(tc.tile_pool(name="ea", bufs=9))
        per = ctx.enter_context(tc.tile_pool(name="er", bufs=4))
        pwts = ctx.enter_context(tc.tile_pool(name="wts", bufs=4))
        pry = ctx.enter_context(tc.tile_pool(name="ry", bufs=2))
        psc = ctx.enter_context(tc.tile_pool(name="sc", bufs=40))
        # PSUM pools — 1+2+1+2+2 = 8 banks
        pp_main = ctx.enter_context(tc.tile_pool(name="ppmain", bufs=1, space="PSUM"))
        pp_l1 = ctx.enter_context(tc.tile_pool(name="ppl1", bufs=2, space="PSUM"))
        pp_l2 = ctx.enter_context(tc.tile_pool(name="ppl2", bufs=2, space="PSUM"))
        pp_sp = ctx.enter_context(tc.tile_pool(name="ppsp", bufs=2, space="PSUM"))
        pp_wav = ctx.enter_context(tc.tile_pool(name="ppwav", bufs=1, space="PSUM"))

        # ---------- constants / weights ----------
        ident = P.tile([128, 128], F32, tag="ident")
        make_identity(nc, ident)
        identB = P.tile([128, 128], BF16, tag="identB")
        make_identity(nc, identB)

        # prologue loads go on the gpsimd (SWDGE) queue so the sync HWDGE ring
        # starts streaming refc groups immediately
        bd1 = P.tile([2 * CIN, 2 * CHID], BF16, tag="bd1")  # [106, 64]
        nc.sync.dma_start(out=bd1, in_=bd1_d[:, :])
        bd2 = P.tile([128, 4 * NH], BF16, tag="bd2")  # [128, 32]
        nc.sync.dma_start(out=bd2, in_=bd2_d[:, :])

        r1b4 = P.tile([128, 1], F32, tag="r1b4")
        for g in range(4):
            nc.sync.dma_start(out=r1b4[32 * g : 32 * g + CHID, :], in_=r1b_d[:, :])

        w_sb = {}
        for nm, d in (("wq", wq_d), ("wk", wk_d), ("wv", wv_d), ("wo", wo_d)):
            for kk in range(NHC):
                t = P.tile([128, H], BF16, tag=f"{nm}{kk}")
                nc.gpsimd.dma_start(out=t, in_=d[128 * kk : 128 * (kk + 1), :])
                w_sb[(nm, kk)] = t

        bias_sb = {}
        for nm, d, has in (("bq", bq_d, has_bq), ("bk", bk_d, has_bk), ("bv", bv_d, has_bv)):
            if has:
                for kk in range(NHC):
                    t = P.tile([128, 1], F32, tag=f"{nm}{kk}")
                    nc.gpsimd.dma_start(out=t, in_=d[128 * kk : 128 * (kk + 1), :])
                    bias_sb[(nm, kk)] = t

        # ---------- mask tiles (natural j order) ----------
        mjb_u8 = P.tile([128, L], U8, tag="mjbu8")
        nc.gpsimd.dma_start(
            out=mjb_u8, in_=bass.AP(tensor=masku8, offset=0, ap=[[0, 128], [1, L]])
        )
        mjb = P.tile([128, L], F32, tag="mjb")
        nc.vector.tensor_copy(out=mjb, in_=mjb_u8)
        # negadd[p, j] = NEG where mask[j]==0 else 0   (same for all partitions)
        negadd = P.tile([128, L], F32, tag="negadd")
        nc.vector.tensor_scalar(
            out=negadd, in0=mjb, scalar1=0.0, scalar2=NEG,
            op0=ALU.is_equal, op1=ALU.mult,
        )
        ones2 = P.tile([128, 128], BF16, tag="ones2")
        nc.gpsimd.memset(ones2, 1.0)
        negj1 = P.tile([1, L], BF16, tag="negj1")
        nc.vector.tensor_copy(out=negj1, in_=negadd[0:1, :])
        negjB = P.tile([128, L], BF16, tag="negjB")
        nc.vector.tensor_copy(out=negjB, in_=negadd)
        ones_f = P.tile([128, H], F32, tag="ones_f")
        nc.gpsimd.memset(ones_f, 1.0)
        eps_t = P.tile([128, 1], F32, tag="eps_t")
        nc.gpsimd.memset(eps_t, 1e-5)

        # per-block invalid-query masks (for the final row fix)
        ivq = []
        for ib in range(NIB):
            miu = P.tile([IBS, 1], U8, tag=f"miu{ib}")
            nc.gpsimd.dma_start(out=miu, in_=ioff_d[IBS * ib : IBS * (ib + 1), :])
            mif = P.tile([IBS, 1], F32, tag=f"mif{ib}")
            nc.vector.tensor_copy(out=mif, in_=miu)
            iv = P.tile([IBS, H], U32, tag=f"ivq{ib}")
            nc.vector.tensor_scalar(
                out=iv, in0=ones_f[0:IBS, :], scalar1=mif, scalar2=0.0,
                op0=ALU.mult, op1=ALU.is_equal,
            )
            ivq.append(iv)

        g_bc = P.tile([128, H], F32, tag="g_bc")
        nc.gpsimd.dma_start(
            out=g_bc, in_=bass.AP(tensor=lng_d, offset=0, ap=[[0, 128], [1, H]])
        )
        b_bc = P.tile([128, H], F32, tag="b_bc")
        nc.gpsimd.dma_start(
            out=b_bc, in_=bass.AP(tensor=lnb_d, offset=0, ap=[[0, 128], [1, H]])
        )
        mvb = P.tile([128, H], F32, tag="mvb")
        nc.gpsimd.dma_start(
            out=mvb, in_=bass.AP(tensor=fixr_d, offset=0, ap=[[0, 128], [1, H]])
        )
        bo_bc = None
        if has_bo:
            bo_bc = P.tile([128, H], F32, tag="bo_bc")
            nc.gpsimd.dma_start(
                out=bo_bc, in_=bass.AP(tensor=bo_d, offset=0, ap=[[0, 128], [1, H]])
            )

        # ---------- x loads, transposes (natural j chunks) ----------
        xf = []
        for t in range(NJB):
            xt = P.tile([128, H], F32, tag=f"xf{t}")
            nc.gpsimd.dma_start(out=xt, in_=xfull[128 * t : 128 * (t + 1), :])
            xf.append(xt)
        xq_sb = []
        for ib in range(NIB):
            xt = P.tile([IBS, H], F32, tag=f"xq{ib}")
            nc.gpsimd.dma_start(out=xt, in_=xq_d[IBS * ib : IBS * (ib + 1), :])
            xq_sb.append(xt)

        evac_rot = [0]

        def evac_copy(out, in_):
            # gpsimd cannot access PSUM on TRN2: rotate scalar/vector only
            r = evac_rot[0] % 2
            evac_rot[0] += 1
            if r == 0:
                nc.scalar.copy(out=out, in_=in_)
            else:
                nc.vector.tensor_copy(out=out, in_=in_)

        xT = []  # [h-chunk][128, L] bf16 — transposed full x (for k, v)
        for hc in range(NHC):
            ps = pp_main.tile([128, L], F32, tag="big")
            for jt in range(NJB):
                nc.tensor.transpose(
                    out=ps[:, 128 * jt : 128 * (jt + 1)],
                    in_=xf[jt][:, 128 * hc : 128 * (hc + 1)],
                    identity=ident,
                )
            xs = P.tile([128, L], BF16, tag=f"xT{hc}")
            evac_copy(xs, ps)
            xT.append(xs)
        xqT = []  # [h-chunk][128, LI] bf16 — transposed xq (for q)
        for hc in range(NHC):
            ps = pp_main.tile([128, LI], F32, tag="big")
            for ib in range(NIB):
                nc.tensor.transpose(
                    out=ps[:, IBS * ib : IBS * (ib + 1)],
                    in_=xq_sb[ib][:, 128 * hc : 128 * (hc + 1)],
                    identity=ident[0:IBS, 0:IBS],
                )
            xs = P.tile([128, LI], BF16, tag=f"xqT{hc}")
            evac_copy(xs, ps)
            xqT.append(xs)

        # ---------- q/k/v projections (bf16) ----------
        qT = []
        for t in range(NHC):
            ps = pp_main.tile([128, LI], F32, tag="big")
            for kk in range(NHC):
                nc.tensor.matmul(
                    out=ps, lhsT=w_sb[("wq", kk)][:, 128 * t : 128 * (t + 1)],
                    rhs=xqT[kk], start=(kk == 0), stop=(kk == NHC - 1),
                )
            s = P.tile([128, LI], BF16, tag=f"qT{t}")
            if has_bq:
                nc.scalar.activation(out=s, in_=ps, func=AF.Identity, bias=bias_sb[("bq", t)])
            else:
                evac_copy(s, ps)
            qT.append(s)
        kT = []
        for t in range(NHC):
            ps = pp_main.tile([128, L], F32, tag="big")
            for kk in range(NHC):
                nc.tensor.matmul(
                    out=ps, lhsT=w_sb[("wk", kk)][:, 128 * t : 128 * (t + 1)],
                    rhs=xT[kk], start=(kk == 0), stop=(kk == NHC - 1),
                )
            s = P.tile([128, L], BF16, tag=f"kT{t}")
            if has_bk:
                nc.scalar.activation(out=s, in_=ps, func=AF.Identity, bias=bias_sb[("bk", t)])
            else:
                evac_copy(s, ps)
            kT.append(s)
        v_sb = []
        for t in range(NJB):
            ps = pp_main.tile([128, H], F32, tag="big")
            for kk in range(NHC):
                nc.tensor.matmul(
                    out=ps, lhsT=xT[kk][:, 128 * t : 128 * (t + 1)],
                    rhs=w_sb[("wv", kk)],
                    start=(kk == 0), stop=(kk == NHC - 1),
                )
            s = P.tile([128, H], BF16, tag=f"v{t}")
            evac_copy(s, ps)  # bv folded into avT evac
            v_sb.append(s)

        # persistent per-head exp-numerator holders for the ref path
        refS = [
            P.tile([IBS, NH, L], BF16, tag=f"refS{ib}", name=f"refS{ib}")
            for ib in range(NIB)
        ]
        aoT = [P.tile([128, LI], BF16, tag=f"aoT{t}", name=f"aoT{t}") for t in range(NHC)]

        # ---------- main per-i-block phases ----------
        for ib in range(NIB):
            # ---- repr-MLP over this i-block's rows ----
            for c in range(NCHUNK):
                # two 128-partition DMAs per 8-pair chunk (rows 106:128 are
                # host zero-pad): the runtime spreads a full-partition dynamic
                # DMA across all 16 SDMA engines
                stgs = []
                for hh in range(2):
                    stg = pstage.tile([128, 4, L], BF16, tag="stg")
                    nc.sync.dma_start(
                        out=stg, in_=refc_d[(ib * NCHUNK + c) * 2 + hh]
                    )
                    stgs.append(stg)
                hids = []
                for gg in range(4):  # 4 groups of 2 pairs each
                    stg = stgs[gg // 2]
                    l1p = pp_l1.tile([128, L], F32, tag="l1")
                    for z in range(2):
                        nc.tensor.matmul(
                            out=l1p[64 * z : 64 * z + 64, :],
                            lhsT=bd1, rhs=stg[0 : 2 * CIN, 2 * (gg % 2) + z, :],
                            start=True, stop=True, tile_position=(0, 64 * z),
                        )
                    hid = phid.tile([128, L], BF16, tag="hid")
                    if gg % 2 == 0:
                        nc.scalar.activation(
                            out=hid, in_=l1p, func=AF.Relu, bias=r1b4,
                        )
                    else:
                        nc.vector.tensor_scalar(
                            out=hid, in0=l1p, scalar1=r1b4, scalar2=0.0,
                            op0=ALU.add, op1=ALU.max,
                        )
                    hids.append(hid)
                l2p = pp_l2.tile([128, L], F32, tag="l2")
                for q in range(4):
                    nc.tensor.matmul(
                        out=l2p[32 * q : 32 * q + 32, :],
                        lhsT=bd2, rhs=hids[q], start=True, stop=False,
                        tile_position=(0, 32 * q), skip_group_check=True,
                    )
                # accumulate the NEG j-mask with a rank-1 matmul so the evac
                # below is a plain (gap-free) scalar copy
                nc.tensor.matmul(
                    out=l2p, lhsT=ones2[0:1, :], rhs=negj1,
                    start=False, stop=True, skip_group_check=True,
                )
                l2s = pl2s.tile([128, L], BF16, tag="l2s")
                nc.scalar.copy(out=l2s, in_=l2p)
                # de-interleave rows -> refS[i, head, :]: l2s partition
                # p = 32q + 8ii + nh pairs 1:1, in natural order, with the
                # (i, nh) runs of refS[16c:16c+16] — a single DMA suffices
                eng = nc.sync if c % 2 == 0 else nc.gpsimd
                eng.dma_start(
                    out=refS[ib][16 * c : 16 * c + 16, :, :],
                    in_=l2s,
                )

            # ---- attention + ref softmax + combine + AV per head ----
            for nh in range(NH):
                t, s = nh // 2, nh % 2
                sp = pp_sp.tile([IBS, L], F32, tag="sp")
                # psum init: 64*NEG on invalid j columns, issued at this
                # head's PE row group so adjacent heads run concurrently
                nc.tensor.matmul(
                    out=sp, lhsT=ones2[64 * s : 64 * s + 64, 0:IBS],
                    rhs=negjB[64 * s : 64 * s + 64, :],
                    start=True, stop=False, skip_group_check=True,
                )
                nc.tensor.matmul(
                    out=sp,
                    lhsT=qT[t][64 * s : 64 * s + 64, IBS * ib : IBS * (ib + 1)],
                    rhs=kT[t][64 * s : 64 * s + 64, :],
                    start=False, stop=True, skip_group_check=True,
                )
                ea_t = pea.tile([IBS, L], BF16, tag="ea")
                sa = psc.tile([IBS, 1], F32, tag="sa")
                nc.scalar.activation(
                    out=ea_t, in_=sp, func=AF.Exp, scale=scale, accum_out=sa
                )

                rt = refS[ib][:, nh, :]
                if has_r2b:
                    nc.vector.tensor_scalar_add(out=rt, in0=rt, scalar1=float(r2b_vals[nh]))
                er_t = per.tile([IBS, L], BF16, tag="er")
                sr = psc.tile([IBS, 1], F32, tag="sr")
                nc.scalar.activation(out=er_t, in_=rt, func=AF.Exp, accum_out=sr)

                isa = psc.tile([IBS, 1], F32, tag="isa")
                nc.vector.reciprocal(out=isa, in_=sa)
                isr = psc.tile([IBS, 1], F32, tag="isr")
                nc.vector.reciprocal(out=isr, in_=sr)
                # w = ea/sa + er/sr (0.5 factor folded into avT evac scale)
                nc.vector.tensor_scalar_mul(out=ea_t, in0=ea_t, scalar1=isa)
                nc.vector.scalar_tensor_tensor(
                    out=ea_t, in0=er_t, scalar=isr, in1=ea_t, op0=ALU.mult, op1=ALU.add
                )

                wtp = pp_wav.tile([128, NJB * IBS], BF16, tag="wav")
                for k in range(NJB):
                    nc.tensor.transpose(
                        out=wtp[:, IBS * k : IBS * (k + 1)],
                        in_=ea_t[:, 128 * k : 128 * (k + 1)],
                        identity=identB[0:IBS, 0:IBS],
                    )
                wts = pwts.tile([128, NJB * IBS], BF16, tag="wts")
                nc.vector.tensor_copy(out=wts, in_=wtp)

                avp = pp_wav.tile([64, IBS], F32, tag="wav")
                for k in range(NJB):
                    nc.tensor.matmul(
                        out=avp,
                        lhsT=v_sb[k][:, 64 * nh : 64 * nh + 64],
                        rhs=wts[:, IBS * k : IBS * (k + 1)],
                        start=(k == 0), stop=(k == NJB - 1),
                    )
                if has_bv:
                    nc.scalar.activation(
                        out=aoT[t][64 * s : 64 * s + 64, IBS * ib : IBS * (ib + 1)],
                        in_=avp, func=AF.Identity, scale=0.5,
                        bias=bias_sb[("bv", t)][64 * s : 64 * s + 64, :],
                    )
                else:
                    nc.scalar.activation(
                        out=aoT[t][64 * s : 64 * s + 64, IBS * ib : IBS * (ib + 1)],
                        in_=avp, func=AF.Copy, bias=0.0, scale=0.5,
                    )

            # ---- output projection + residual + row fix + layernorm ----
            pp = pp_main.tile([IBS, H], F32, tag="big")
            for kk in range(NHC):
                nc.tensor.matmul(
                    out=pp, lhsT=aoT[kk][:, IBS * ib : IBS * (ib + 1)],
                    rhs=w_sb[("wo", kk)],
                    start=(kk == 0), stop=(kk == NHC - 1),
                )
            r_t = pry.tile([IBS, H], F32, tag="rt")
            nc.vector.scalar_tensor_tensor(
                out=r_t, in0=pp, scalar=1.0, in1=xq_sb[ib], op0=ALU.mult, op1=ALU.add
            )
            # invalid query rows get the exact uniform-attention result
            fr = pry.tile([IBS, H], F32, tag="fr")
            nc.vector.tensor_add(out=fr, in0=xq_sb[ib], in1=mvb[0:IBS, :])
            nc.vector.copy_predicated(out=r_t, mask=ivq[ib], data=fr)
            if has_bo:
                nc.vector.tensor_add(out=r_t, in0=r_t, in1=bo_bc[0:IBS, :])
            stats = psc.tile([IBS, 6], F32, tag="stats")
            nc.vector.bn_stats(out=stats, in_=r_t)
            mv = psc.tile([IBS, 2], F32, tag="mv")
            nc.vector.bn_aggr(out=mv, in_=stats)
            stdv = psc.tile([IBS, 1], F32, tag="stdv")
            nc.scalar.activation(out=stdv, in_=mv[:, 1:2], func=AF.Sqrt, bias=eps_t[0:IBS, :])
            rstd = psc.tile([IBS, 1], F32, tag="rstd")
            nc.vector.reciprocal(out=rstd, in_=stdv)
            nc.vector.tensor_scalar(
                out=r_t, in0=r_t, scalar1=mv[:, 0:1], scalar2=rstd,
                op0=ALU.subtract, op1=ALU.mult,
            )
            y_t = pry.tile([IBS, H], F32, tag="yt")
            nc.vector.tensor_mul(out=y_t, in0=r_t, in1=g_bc[0:IBS, :])
            nc.vector.tensor_add(out=y_t, in0=y_t, in1=b_bc[0:IBS, :])
            nc.scalar.dma_start(out=y_d[IBS * ib : IBS * (ib + 1), :], in_=y_t)

    nc.compile()  # Bacc legalization: ≤1 sync wait per instruction, etc.
    return nc


def _make_bd1(r1w):
    bd1 = np.zeros((2 * CIN, 2 * CHID), np.float32)
    bd1[0:CIN, 0:CHID] = r1w
    bd1[CIN : 2 * CIN, CHID : 2 * CHID] = r1w
    return bd1.astype(NPBF16)


def _make_bd2(r2w):
    bd2 = np.zeros((128, 4 * NH), np.float32)
    for g in range(4):
        bd2[32 * g : 32 * g + CHID, NH * g : NH * (g + 1)] = r2w
    return bd2.astype(NPBF16)


def _pack_refc(refc_core, maski):
    """[LI, L, CIN] f32 + [LI] bool -> pair-packed transposed [LI//2, 2*CIN, L] bf16.

    Invalid query rows are zeroed so their (masked) softmax stays finite and
    row-constant; those output rows are overwritten by the uniform fix anyway.
    """
    LIc, Lc, _ = refc_core.shape
    rc = np.asarray(refc_core, np.float32) * np.asarray(maski, np.float32)[:, None, None]
    rc = rc.astype(NPBF16)
    rc = rc.reshape(LIc // 2, 2, Lc, CIN).transpose(0, 1, 3, 2)
    rc = rc.reshape(LIc // 2, 2 * CIN, Lc)
    # 4-pair chunks: [C, 128, 4, L] so each load is one contiguous
    # 4KB-per-partition descriptor chain over all 128 partitions (the runtime
    # only engages all 16 SDMA engines for full-partition transfers)
    rc = rc.reshape(LIc // 8, 4, 2 * CIN, Lc).transpose(0, 2, 1, 3)
    out = np.zeros((LIc // 8, 128, 4, Lc), NPBF16)
    out[:, 0 : 2 * CIN] = rc
    return out


_PROG_CACHE = {}


def _get_program(L, LI, flags, r2b_vals):
    key = (L, LI, flags)
    if key not in _PROG_CACHE:
        _PROG_CACHE[key] = build_program(L, LI, *flags, r2b_vals)
    return _PROG_CACHE[key]


def make_in_maps(x, mask, refCov, wq, bq, wk, bk, wv, bv, wo, bo,
                 r1w, r1b, r2w, r2b, ln_g, ln_b, n_cores=N_CORES, LI=None):
    Bc, L, Hc = x.shape
    if LI is None:
        LI = (Bc * L) // n_cores
    f = np.float32
    shared = {
        "wq": np.asarray(wq, f).astype(NPBF16),
        "wk": np.asarray(wk, f).astype(NPBF16),
        "wv": np.asarray(wv, f).astype(NPBF16),
        "wo": np.asarray(wo, f).astype(NPBF16),
        "bq": np.ascontiguousarray(bq, f).reshape(Hc, 1),
        "bk": np.ascontiguousarray(bk, f).reshape(Hc, 1),
        "bv": np.ascontiguousarray(bv, f).reshape(Hc, 1),
        "bo": np.ascontiguousarray(bo, f).reshape(Hc, 1),
        "bd1h": _make_bd1(np.asarray(r1w, f)),
        "bd2h": _make_bd2(np.asarray(r2w, f)),
        "r1b": np.ascontiguousarray(r1b, f).reshape(CHID, 1),
        "lng": np.ascontiguousarray(ln_g, f).reshape(Hc, 1),
        "lnb": np.ascontiguousarray(ln_b, f).reshape(Hc, 1),
    }
    per_batch = L // LI  # cores per batch
    in_maps = []
    for c in range(n_cores):
        b, half = c // per_batch, c % per_batch
        i0 = half * LI
        m = dict(shared)
        m["xfull"] = np.ascontiguousarray(x[b], f)
        m["xq"] = np.ascontiguousarray(x[b, i0 : i0 + LI], f)
        m["refc"] = _pack_refc(refCov[b, i0 : i0 + LI], mask[b, i0 : i0 + LI])
        m["masku8"] = np.ascontiguousarray(mask[b].astype(np.uint8).reshape(L, 1))
        m["ioff"] = np.ascontiguousarray(
            mask[b, i0 : i0 + LI].astype(np.uint8).reshape(LI, 1)
        )
        # uniform-attention output row for fully-masked queries:
        # mean_j(v) @ wo  (bias bo is added on-device for all rows)
        mean_v = np.asarray(x[b], f).mean(axis=0) @ np.asarray(wv, f) + np.asarray(bv, f)
        m["fixrow"] = np.ascontiguousarray((mean_v @ np.asarray(wo, f)).reshape(Hc, 1), f)
        negrow = np.where(mask[b], 0.0, -1.0e30).astype(NPBF16)
        m["negpre"] = np.ascontiguousarray(np.tile(negrow, NH).reshape(NH * L, 1))
        in_maps.append(m)
    return in_maps, per_batch, LI


def kernel(x, mask, refCov, wq, bq, wk, bk, wv, bv, wo, bo,
           r1w, r1b, r2w, r2b, ln_g, ln_b, trace=False):
    x = np.asarray(x)
    Bc, L, Hc = x.shape
    flags = (
        bool(np.any(bq)), bool(np.any(bk)), bool(np.any(bv)), bool(np.any(bo)),
        bool(np.any(r2b)),
    )
    in_maps, per_batch, LI = make_in_maps(
        x, mask, refCov, wq, bq, wk, bk, wv, bv, wo, bo,
        r1w, r1b, r2w, r2b, ln_g, ln_b,
    )
    nc = _get_program(L, LI, flags, [float(v) for v in np.asarray(r2b).ravel()])
    res = run_bass_kernel_spmd(nc, in_maps, core_ids=list(range(N_CORES)), trace=trace)
    out = np.empty((Bc, L, Hc), np.float32)
    for c in range(N_CORES):
        b, half = c // per_batch, c % per_batch
        out[b, half * LI : (half + 1) * LI] = res.results[c]["y"]
    if trace:
        return out, res
    return out


# revision 35
# speedup vs baseline: 1.0240x; 1.0184x over previous
"""Trainium2 Bass kernel for nn_AttentionBlock (sparse_attention).

Full-input contract: kernel(**inputs) takes the complete tensors and returns
the complete [4, 512, 512] output. Internally shards over 8 NeuronCores as
(batch, i-half): core c handles batch c//2, query rows (c%2)*256 ..+256.

Key layout choice: refCov is transposed host-side into pair-packed
[NPAIR, 2*53, L] bf16 so the device L1 matmul consumes it directly with the
feature dim on partitions (no on-device repack/transpose). Masking is
additive (-1e30 on invalid j columns, folded into the L2 evacuation and a
rank-1 PSUM init for scores); rows with an invalid query index are fixed at
the end with a host-precomputed uniform-attention row.

Self-contained: hardcodes all shapes; no sibling imports.
"""

import sys

if "/opt/trn_rl_repo" not in sys.path:
    sys.path.insert(0, "/opt/trn_rl_repo")

from contextlib import ExitStack

import numpy as np
import ml_dtypes

import concourse.bass as bass
import concourse.tile as tile
from concourse import bacc, mybir
from concourse.bass_utils import run_bass_kernel_spmd
from concourse.masks import make_identity

F32 = mybir.dt.float32
BF16 = mybir.dt.bfloat16
U8 = mybir.dt.uint8
U32 = mybir.dt.uint32
AF = mybir.ActivationFunctionType
ALU = mybir.AluOpType
AX = mybir.AxisListType

NPBF16 = ml_dtypes.bfloat16

NEG = -1.0e30

# Full-problem constants
B, L_FULL, H, NH = 4, 512, 512, 8
DK = H // NH  # 64
CIN, CHID = 53, 32  # repr MLP dims
N_CORES = 8


def build_program(L, LI, has_bq, has_bk, has_bv, has_bo, has_r2b, r2b_vals,
                  trace_sim=False):
    """One-core program: attention block over LI query rows, L context.

    L multiple of 128; LI multiple of 16 (and of 128 when >= 128).
    r2b_vals: python floats baked as immediates when has_r2b.
    """
    assert L % 128 == 0 and LI % 16 == 0
    NJB = L // 128            # j blocks (k-chunks of v / wT)
    NPAIR = LI // 2           # i-row pairs (layer-1 packing)
    IBS = min(128, LI)        # i-block size for attention tiles
    NIB = LI // IBS           # i-blocks
    NHC = H // 128            # h chunks (4)
    assert (IBS // 2) % 8 == 0
    NCHUNK = IBS // 16        # 16-i-row chunks per i-block (4 groups of 2 pairs)
    scale = float(1.0 / np.sqrt(DK))

    nc = bacc.Bacc()

    xfull = nc.dram_tensor("xfull", [L, H], F32, kind="ExternalInput")
    xq_d = nc.dram_tensor("xq", [LI, H], F32, kind="ExternalInput")
    refc_d = nc.dram_tensor("refc", [NPAIR // 4, 128, 4, L], BF16, kind="ExternalInput")
    masku8 = nc.dram_tensor("masku8", [L, 1], U8, kind="ExternalInput")
    ioff_d = nc.dram_tensor("ioff", [LI, 1], U8, kind="ExternalInput")
    wq_d = nc.dram_tensor("wq", [H, H], BF16, kind="ExternalInput")
    wk_d = nc.dram_tensor("wk", [H, H], BF16, kind="ExternalInput")
    wv_d = nc.dram_tensor("wv", [H, H], BF16, kind="ExternalInput")
    wo_d = nc.dram_tensor("wo", [H, H], BF16, kind="ExternalInput")
    bq_d = nc.dram_tensor("bq", [H, 1], F32, kind="ExternalInput")
    bk_d = nc.dram_tensor("bk", [H, 1], F32, kind="ExternalInput")
    bv_d = nc.dram_tensor("bv", [H, 1], F32, kind="ExternalInput")
    bo_d = nc.dram_tensor("bo", [H, 1], F32, kind="ExternalInput")
    bd1_d = nc.dram_tensor("bd1h", [2 * CIN, 2 * CHID], BF16, kind="ExternalInput")
    bd2_d = nc.dram_tensor("bd2h", [128, 4 * NH], BF16, kind="ExternalInput")
    r1b_d = nc.dram_tensor("r1b", [CHID, 1], F32, kind="ExternalInput")
    fixr_d = nc.dram_tensor("fixrow", [H, 1], F32, kind="ExternalInput")
    negpre_d = nc.dram_tensor("negpre", [NH * L, 1], BF16, kind="ExternalInput")
    lng_d = nc.dram_tensor("lng", [H, 1], F32, kind="ExternalInput")
    lnb_d = nc.dram_tensor("lnb", [H, 1], F32, kind="ExternalInput")
    y_d = nc.dram_tensor("y", [LI, H], F32, kind="ExternalOutput")

    with tile.TileContext(nc, trace_sim=trace_sim) as tc, ExitStack() as ctx:
        P = ctx.enter_context(tc.tile_pool(name="persist", bufs=1))
        pstage = ctx.enter_context(tc.tile_pool(name="stage", bufs=6))
        phid = ctx.enter_context(tc.tile_pool(name="hid", bufs=6))
        pl2s = ctx.enter_context(tc.tile_pool(name="l2s", bufs=3))
        pea = ctx.enter_context(tc.tile_pool(name="ea", bufs=9))
        per = ctx.enter_context(tc.tile_pool(name="er", bufs=4))
        pwts = ctx.enter_context(tc.tile_pool(name="wts", bufs=4))
        pry = ctx.enter_context(tc.tile_pool(name="ry", bufs=2))
        psc = ctx.enter_context(tc.tile_pool(name="sc", bufs=40))
        # PSUM pools — 1+2+1+2+2 = 8 banks
        pp_main = ctx.enter_context(tc.tile_pool(name="ppmain", bufs=1, space="PSUM"))
        pp_l1 = ctx.enter_context(tc.tile_pool(name="ppl1", bufs=2, space="PSUM"))
        pp_l2 = ctx.enter_context(tc.tile_pool(name="ppl2", bufs=2, space="PSUM"))
        pp_sp = ctx.enter_context(tc.tile_pool(name="ppsp", bufs=2, space="PSUM"))
        pp_wav = ctx.enter_context(tc.tile_pool(name="ppwav", bufs=1, space="PSUM"))

        # ---------- constants / weights ----------
        ident = P.tile([128, 128], F32, tag="ident")
        make_identity(nc, ident)
        identB = P.tile([128, 128], BF16, tag="identB")
        make_identity(nc, identB)

        # prologue loads go on the gpsimd (SWDGE) queue so the sync HWDGE ring
        # starts streaming refc groups immediately
        bd1 = P.tile([2 * CIN, 2 * CHID], BF16, tag="bd1")  # [106, 64]
        nc.sync.dma_start(out=bd1, in_=bd1_d[:, :])
        bd2 = P.tile([128, 4 * NH], BF16, tag="bd2")  # [128, 32]
        nc.sync.dma_start(out=bd2, in_=bd2_d[:, :])

        r1b4 = P.tile([128, 1], F32, tag="r1b4")
        for g in range(4):
            nc.sync.dma_start(out=r1b4[32 * g : 32 * g + CHID, :], in_=r1b_d[:, :])

        w_sb = {}
        for nm, d in (("wq", wq_d), ("wk", wk_d), ("wv", wv_d), ("wo", wo_d)):
            for kk in range(NHC):
                t = P.tile([128, H], BF16, tag=f"{nm}{kk}")
                nc.gpsimd.dma_start(out=t, in_=d[128 * kk : 128 * (kk + 1), :])
                w_sb[(nm, kk)] = t

        bias_sb = {}
        for nm, d, has in (("bq", bq_d, has_bq), ("bk", bk_d, has_bk), ("bv", bv_d, has_bv)):
            if has:
                for kk in range(NHC):
                    t = P.tile([128, 1], F32, tag=f"{nm}{kk}")
                    nc.gpsimd.dma_start(out=t, in_=d[128 * kk : 128 * (kk + 1), :])
                    bias_sb[(nm, kk)] = t

        # ---------- mask tiles (natural j order) ----------
        mjb_u8 = P.tile([128, L], U8, tag="mjbu8")
        nc.gpsimd.dma_start(
            out=mjb_u8, in_=bass.AP(tensor=masku8, offset=0, ap=[[0, 128], [1, L]])
        )
        mjb = P.tile([128, L], F32, tag="mjb")
        nc.vector.tensor_copy(out=mjb, in_=mjb_u8)
        # negadd[p, j] = NEG where mask[j]==0 else 0   (same for all partitions)
        negadd = P.tile([128, L], F32, tag="negadd")
        nc.vector.tensor_scalar(
            out=negadd, in0=mjb, scalar1=0.0, scalar2=NEG,
            op0=ALU.is_equal, op1=ALU.mult,
        )
        ones2 = P.tile([128, 128], BF16, tag="ones2")
        nc.gpsimd.memset(ones2, 1.0)
        negj1 = P.tile([1, L], BF16, tag="negj1")
        nc.vector.tensor_copy(out=negj1, in_=negadd[0:1, :])
        negjB = P.tile([128, L], BF16, tag="negjB")
        nc.vector.tensor_copy(out=negjB, in_=negadd)
        ones_f = P.tile([128, H], F32, tag="ones_f")
        nc.gpsimd.memset(ones_f, 1.0)
        eps_t = P.tile([128, 1], F32, tag="eps_t")
        nc.gpsimd.memset(eps_t, 1e-5)

        # per-block invalid-query masks (for the final row fix)
        ivq = []
        for ib in range(NIB):
            miu = P.tile([IBS, 1], U8, tag=f"miu{ib}")
            nc.gpsimd.dma_start(out=miu, in_=ioff_d[IBS * ib : IBS * (ib + 1), :])
            mif = P.tile([IBS, 1], F32, tag=f"mif{ib}")
            nc.vector.tensor_copy(out=mif, in_=miu)
            iv = P.tile([IBS, H], U32, tag=f"ivq{ib}")
            nc.vector.tensor_scalar(
                out=iv, in0=ones_f[0:IBS, :], scalar1=mif, scalar2=0.0,
                op0=ALU.mult, op1=ALU.is_equal,
            )
            ivq.append(iv)

        g_bc = P.tile([128, H], F32, tag="g_bc")
        nc.gpsimd.dma_start(
            out=g_bc, in_=bass.AP(tensor=lng_d, offset=0, ap=[[0, 128], [1, H]])
        )
        b_bc = P.tile([128, H], F32, tag="b_bc")
        nc.gpsimd.dma_start(
            out=b_bc, in_=bass.AP(tensor=lnb_d, offset=0, ap=[[0, 128], [1, H]])
        )
        mvb = P.tile([128, H], F32, tag="mvb")
        nc.gpsimd.dma_start(
            out=mvb, in_=bass.AP(tensor=fixr_d, offset=0, ap=[[0, 128], [1, H]])
        )
        bo_bc = None
        if has_bo:
            bo_bc = P.tile([128, H], F32, tag="bo_bc")
            nc.gpsimd.dma_start(
                out=bo_bc, in_=bass.AP(tensor=bo_d, offset=0, ap=[[0, 128], [1, H]])
            )

        # ---------- x loads, transposes (natural j chunks) ----------
        xf = []
        for t in range(NJB):
            xt = P.tile([128, H], F32, tag=f"xf{t}")
            nc.gpsimd.dma_start(out=xt, in_=xfull[128 * t : 128 * (t + 1), :])
            xf.append(xt)
        xq_sb = []
        for ib in range(NIB):
            xt = P.tile([IBS, H], F32, tag=f"xq{ib}")
            nc.gpsimd.dma_start(out=xt, in_=xq_d[IBS * ib : IBS * (ib + 1), :])
            xq_sb.append(xt)

        evac_rot = [0]

        def evac_copy(out, in_):
            # gpsimd cannot access PSUM on TRN2: rotate scalar/vector only
            r = evac_rot[0] % 2
            evac_rot[0] += 1
            if r == 0:
                nc.scalar.copy(out=out, in_=in_)
            else:
                nc.vector.tensor_copy(out=out, in_=in_)

        xT = []  # [h-chunk][128, L] bf16 — transposed full x (for k, v)
        for hc in range(NHC):
            ps = pp_main.tile([128, L], F32, tag="big")
            for jt in range(NJB):
                nc.tensor.transpose(
                    out=ps[:, 128 * jt : 128 * (jt + 1)],
                    in_=xf[jt][:, 128 * hc : 128 * (hc + 1)],
                    identity=ident,
                )
            xs = P.tile([128, L], BF16, tag=f"xT{hc}")
            evac_copy(xs, ps)
            xT.append(xs)
        xqT = []  # [h-chunk][128, LI] bf16 — transposed xq (for q)
        for hc in range(NHC):
            ps = pp_main.tile([128, LI], F32, tag="big")
            for ib in range(NIB):
                nc.tensor.transpose(
                    out=ps[:, IBS * ib : IBS * (ib + 1)],
                    in_=xq_sb[ib][:, 128 * hc : 128 * (hc + 1)],
                    identity=ident[0:IBS, 0:IBS],
                )
            xs = P.tile([128, LI], BF16, tag=f"xqT{hc}")
            evac_copy(xs, ps)
            xqT.append(xs)

        # ---------- q/k/v projections (bf16) ----------
        qT = []
        for t in range(NHC):
            ps = pp_main.tile([128, LI], F32, tag="big")
            for kk in range(NHC):
                nc.tensor.matmul(
                    out=ps, lhsT=w_sb[("wq", kk)][:, 128 * t : 128 * (t + 1)],
                    rhs=xqT[kk], start=(kk == 0), stop=(kk == NHC - 1),
                )
            s = P.tile([128, LI], BF16, tag=f"qT{t}")
            if has_bq:
                nc.scalar.activation(out=s, in_=ps, func=AF.Identity, bias=bias_sb[("bq", t)])
            else:
                evac_copy(s, ps)
            qT.append(s)
        kT = []
        for t in range(NHC):
            ps = pp_main.tile([128, L], F32, tag="big")
            for kk in range(NHC):
                nc.tensor.matmul(
                    out=ps, lhsT=w_sb[("wk", kk)][:, 128 * t : 128 * (t + 1)],
                    rhs=xT[kk], start=(kk == 0), stop=(kk == NHC - 1),
                )
            s = P.tile([128, L], BF16, tag=f"kT{t}")
            if has_bk:
                nc.scalar.activation(out=s, in_=ps, func=AF.Identity, bias=bias_sb[("bk", t)])
            else:
                evac_copy(s, ps)
            kT.append(s)
        v_sb = []
        for t in range(NJB):
            ps = pp_main.tile([128, H], F32, tag="big")
            for kk in range(NHC):
                nc.tensor.matmul(
                    out=ps, lhsT=xT[kk][:, 128 * t : 128 * (t + 1)],
                    rhs=w_sb[("wv", kk)],
                    start=(kk == 0), stop=(kk == NHC - 1),
                )
            s = P.tile([128, H], BF16, tag=f"v{t}")
            evac_copy(s, ps)  # bv folded into avT evac
            v_sb.append(s)

        # persistent per-head exp-numerator holders for the ref path
        refS = [
            P.tile([IBS, NH, L], BF16, tag=f"refS{ib}", name=f"refS{ib}")
            for ib in range(NIB)
        ]
        aoT = [P.tile([128, LI], BF16, tag=f"aoT{t}", name=f"aoT{t}") for t in range(NHC)]

        # ---------- main per-i-block phases ----------
        for ib in range(NIB):
            # ---- repr-MLP over this i-block's rows ----
            for c in range(NCHUNK):
                # two 128-partition DMAs per 8-pair chunk (rows 106:128 are
                # host zero-pad): the runtime spreads a full-partition dynamic
                # DMA across all 16 SDMA engines
                stgs = []
                for hh in range(2):
                    stg = pstage.tile([128, 4, L], BF16, tag="stg")
                    nc.sync.dma_start(
                        out=stg, in_=refc_d[(ib * NCHUNK + c) * 2 + hh]
                    )
                    stgs.append(stg)
                hids = []
                for gg in range(4):  # 4 groups of 2 pairs each
                    stg = stgs[gg // 2]
                    l1p = pp_l1.tile([128, L], F32, tag="l1")
                    for z in range(2):
                        nc.tensor.matmul(
                            out=l1p[64 * z : 64 * z + 64, :],
                            lhsT=bd1, rhs=stg[0 : 2 * CIN, 2 * (gg % 2) + z, :],
                            start=True, stop=True, tile_position=(0, 64 * z),
                        )
                    hid = phid.tile([128, L], BF16, tag="hid")
                    if gg % 2 == 0:
                        nc.scalar.activation(
                            out=hid, in_=l1p, func=AF.Relu, bias=r1b4,
                        )
                    else:
                        nc.vector.tensor_scalar(
                            out=hid, in0=l1p, scalar1=r1b4, scalar2=0.0,
                            op0=ALU.add, op1=ALU.max,
                        )
                    hids.append(hid)
                l2p = pp_l2.tile([128, L], F32, tag="l2")
                for q in range(4):
                    nc.tensor.matmul(
                        out=l2p[32 * q : 32 * q + 32, :],
                        lhsT=bd2, rhs=hids[q], start=True, stop=True,
                        tile_position=(0, 32 * q),
                    )
                # masked exp-numerator input: raw + NEG on invalid j columns
                l2s = pl2s.tile([128, L], BF16, tag="l2s")
                nc.vector.tensor_add(out=l2s, in0=l2p, in1=negadd)
                # de-interleave rows -> refS[i, head, :]: l2s partition
                # p = 32q + 8ii + nh pairs 1:1, in natural order, with the
                # (i, nh) runs of refS[16c:16c+16] — a single DMA suffices
                eng = nc.sync if c % 2 == 0 else nc.gpsimd
                eng.dma_start(
                    out=refS[ib][16 * c : 16 * c + 16, :, :],
                    in_=l2s,
                )

            # ---- attention + ref softmax + combine + AV per head ----
            for nh in range(NH):
                t, s = nh // 2, nh % 2
                sp = pp_sp.tile([IBS, L], F32, tag="sp")
                # psum init: 64*NEG on invalid j columns, issued at this
                # head's PE row group so adjacent heads run concurrently
                nc.tensor.matmul(
                    out=sp, lhsT=ones2[64 * s : 64 * s + 64, 0:IBS],
                    rhs=negjB[64 * s : 64 * s + 64, :],
                    start=True, stop=False, skip_group_check=True,
                )
                nc.tensor.matmul(
                    out=sp,
                    lhsT=qT[t][64 * s : 64 * s + 64, IBS * ib : IBS * (ib + 1)],
                    rhs=kT[t][64 * s : 64 * s + 64, :],
                    start=False, stop=True, skip_group_check=True,
                )
                ea_t = pea.tile([IBS, L], BF16, tag="ea")
                sa = psc.tile([IBS, 1], F32, tag="sa")
                nc.scalar.activation(
                    out=ea_t, in_=sp, func=AF.Exp, scale=scale, accum_out=sa
                )

                rt = refS[ib][:, nh, :]
                if has_r2b:
                    nc.vector.tensor_scalar_add(out=rt, in0=rt, scalar1=float(r2b_vals[nh]))
                er_t = per.tile([IBS, L], BF16, tag="er")
                sr = psc.tile([IBS, 1], F32, tag="sr")
                nc.scalar.activation(out=er_t, in_=rt, func=AF.Exp, accum_out=sr)

                isa = psc.tile([IBS, 1], F32, tag="isa")
                nc.vector.reciprocal(out=isa, in_=sa)
                isr = psc.tile([IBS, 1], F32, tag="isr")
                nc.vector.reciprocal(out=isr, in_=sr)
                # w = ea/sa + er/sr (0.5 factor folded into avT evac scale)
                nc.vector.tensor_scalar_mul(out=ea_t, in0=ea_t, scalar1=isa)
                nc.vector.scalar_tensor_tensor(
                    out=ea_t, in0=er_t, scalar=isr, in1=ea_t, op0=ALU.mult, op1=ALU.add
                )

                wtp = pp_wav.tile([128, NJB * IBS], BF16, tag="wav")
                for k in range(NJB):
                    nc.tensor.transpose(
                        out=wtp[:, IBS * k : IBS * (k + 1)],
                        in_=ea_t[:, 128 * k : 128 * (k + 1)],
                        identity=identB[0:IBS, 0:IBS],
                    )
                wts = pwts.tile([128, NJB * IBS], BF16, tag="wts")
                nc.vector.tensor_copy(out=wts, in_=wtp)

                avp = pp_wav.tile([64, IBS], F32, tag="wav")
                for k in range(NJB):
                    nc.tensor.matmul(
                        out=avp,
                        lhsT=v_sb[k][:, 64 * nh : 64 * nh + 64],
                        rhs=wts[:, IBS * k : IBS * (k + 1)],
                        start=(k == 0), stop=(k == NJB - 1),
                    )
                if has_bv:
                    nc.scalar.activation(
                        out=aoT[t][64 * s : 64 * s + 64, IBS * ib : IBS * (ib + 1)],
                        in_=avp, func=AF.Identity, scale=0.5,
                        bias=bias_sb[("bv", t)][64 * s : 64 * s + 64, :],
                    )
                else:
                    nc.scalar.activation(
                        out=aoT[t][64 * s : 64 * s + 64, IBS * ib : IBS * (ib + 1)],
                        in_=avp, func=AF.Copy, bias=0.0, scale=0.5,
                    )

            # ---- output projection + residual + row fix + layernorm ----
            pp = pp_main.tile([IBS, H], F32, tag="big")
            for kk in range(NHC):
                nc.tensor.matmul(
                    out=pp, lhsT=aoT[kk][:, IBS * ib : IBS * (ib + 1)],
                    rhs=w_sb[("wo", kk)],
                    start=(kk == 0), stop=(kk == NHC - 1),
                )
            r_t = pry.tile([IBS, H], F32, tag="rt")
            nc.vector.scalar_tensor_tensor(
                out=r_t, in0=pp, scalar=1.0, in1=xq_sb[ib], op0=ALU.mult, op1=ALU.add
            )
            # invalid query rows get the exact uniform-attention result
            fr = pry.tile([IBS, H], F32, tag="fr")
            nc.vector.tensor_add(out=fr, in0=xq_sb[ib], in1=mvb[0:IBS, :])
            nc.vector.copy_predicated(out=r_t, mask=ivq[ib], data=fr)
            if has_bo:
                nc.vector.tensor_add(out=r_t, in0=r_t, in1=bo_bc[0:IBS, :])
            stats = psc.tile([IBS, 6], F32, tag="stats")
            nc.vector.bn_stats(out=stats, in_=r_t)
            mv = psc.tile([IBS, 2], F32, tag="mv")
            nc.vector.bn_aggr(out=mv, in_=stats)
            stdv = psc.tile([IBS, 1], F32, tag="stdv")
            nc.scalar.activation(out=stdv, in_=mv[:, 1:2], func=AF.Sqrt, bias=eps_t[0:IBS, :])
            rstd = psc.tile([IBS, 1], F32, tag="rstd")
            nc.vector.reciprocal(out=rstd, in_=stdv)
            nc.vector.tensor_scalar(
                out=r_t, in0=r_t, scalar1=mv[:, 0:1], scalar2=rstd,
                op0=ALU.subtract, op1=ALU.mult,
            )
            y_t = pry.tile([IBS, H], F32, tag="yt")
            nc.vector.tensor_mul(out=y_t, in0=r_t, in1=g_bc[0:IBS, :])
            nc.vector.tensor_add(out=y_t, in0=y_t, in1=b_bc[0:IBS, :])
            nc.scalar.dma_start(out=y_d[IBS * ib : IBS * (ib + 1), :], in_=y_t)

    nc.compile()  # Bacc legalization: ≤1 sync wait per instruction, etc.
    return nc


def _make_bd1(r1w):
    bd1 = np.zeros((2 * CIN, 2 * CHID), np.float32)
    bd1[0:CIN, 0:CHID] = r1w
    bd1[CIN : 2 * CIN, CHID : 2 * CHID] = r1w
    return bd1.astype(NPBF16)


def _make_bd2(r2w):
    bd2 = np.zeros((128, 4 * NH), np.float32)
    for g in range(4):
        bd2[32 * g : 32 * g + CHID, NH * g : NH * (g + 1)] = r2w
    return bd2.astype(NPBF16)


def _pack_refc(refc_core, maski):
    """[LI, L, CIN] f32 + [LI] bool -> pair-packed transposed [LI//2, 2*CIN, L] bf16.

    Invalid query rows are zeroed so their (masked) softmax stays finite and
    row-constant; those output rows are overwritten by the uniform fix anyway.
    """
    LIc, Lc, _ = refc_core.shape
    rc = np.asarray(refc_core, np.float32) * np.asarray(maski, np.float32)[:, None, None]
    rc = rc.astype(NPBF16)
    rc = rc.reshape(LIc // 2, 2, Lc, CIN).transpose(0, 1, 3, 2)
    rc = rc.reshape(LIc // 2, 2 * CIN, Lc)
    # 4-pair chunks: [C, 128, 4, L] so each load is one contiguous
    # 4KB-per-partition descriptor chain over all 128 partitions (the runtime
    # only engages all 16 SDMA engines for full-partition transfers)
    rc = rc.reshape(LIc // 8, 4, 2 * CIN, Lc).transpose(0, 2, 1, 3)
    out = np.zeros((LIc // 8, 128, 4, Lc), NPBF16)
    out[:, 0 : 2 * CIN] = rc
    return out


_PROG_CACHE = {}


def _get_program(L, LI, flags, r2b_vals):
    key = (L, LI, flags)
    if key not in _PROG_CACHE:
        _PROG_CACHE[key] = build_program(L, LI, *flags, r2b_vals)
    return _PROG_CACHE[key]


def make_in_maps(x, mask, refCov, wq, bq, wk, bk, wv, bv, wo, bo,
                 r1w, r1b, r2w, r2b, ln_g, ln_b, n_cores=N_CORES, LI=None):
    Bc, L, Hc = x.shape
    if LI is None:
        LI = (Bc * L) // n_cores
    f = np.float32
    shared = {
        "wq": np.asarray(wq, f).astype(NPBF16),
        "wk": np.asarray(wk, f).astype(NPBF16),
        "wv": np.asarray(wv, f).astype(NPBF16),
        "wo": np.asarray(wo, f).astype(NPBF16),
        "bq": np.ascontiguousarray(bq, f).reshape(Hc, 1),
        "bk": np.ascontiguousarray(bk, f).reshape(Hc, 1),
        "bv": np.ascontiguousarray(bv, f).reshape(Hc, 1),
        "bo": np.ascontiguousarray(bo, f).reshape(Hc, 1),
        "bd1h": _make_bd1(np.asarray(r1w, f)),
        "bd2h": _make_bd2(np.asarray(r2w, f)),
        "r1b": np.ascontiguousarray(r1b, f).reshape(CHID, 1),
        "lng": np.ascontiguousarray(ln_g, f).reshape(Hc, 1),
        "lnb": np.ascontiguousarray(ln_b, f).reshape(Hc, 1),
    }
    per_batch = L // LI  # cores per batch
    in_maps = []
    for c in range(n_cores):
        b, half = c // per_batch, c % per_batch
        i0 = half * LI
        m = dict(shared)
        m["xfull"] = np.ascontiguousarray(x[b], f)
        m["xq"] = np.ascontiguousarray(x[b, i0 : i0 + LI], f)
        m["refc"] = _pack_refc(refCov[b, i0 : i0 + LI], mask[b, i0 : i0 + LI])
        m["masku8"] = np.ascontiguousarray(mask[b].astype(np.uint8).reshape(L, 1))
        m["ioff"] = np.ascontiguousarray(
            mask[b, i0 : i0 + LI].astype(np.uint8).reshape(LI, 1)
        )
        # uniform-attention output row for fully-masked queries:
        # mean_j(v) @ wo  (bias bo is added on-device for all rows)
        mean_v = np.asarray(x[b], f).mean(axis=0) @ np.asarray(wv, f) + np.asarray(bv, f)
        m["fixrow"] = np.ascontiguousarray((mean_v @ np.asarray(wo, f)).reshape(Hc, 1), f)
        negrow = np.where(mask[b], 0.0, -1.0e30).astype(NPBF16)
        m["negpre"] = np.ascontiguousarray(np.tile(negrow, NH).reshape(NH * L, 1))
        in_maps.append(m)
    return in_maps, per_batch, LI


def kernel(x, mask, refCov, wq, bq, wk, bk, wv, bv, wo, bo,
           r1w, r1b, r2w, r2b, ln_g, ln_b, trace=False):
    x = np.asarray(x)
    Bc, L, Hc = x.shape
    flags = (
        bool(np.any(bq)), bool(np.any(bk)), bool(np.any(bv)), bool(np.any(bo)),
        bool(np.any(r2b)),
    )
    in_maps, per_batch, LI = make_in_maps(
        x, mask, refCov, wq, bq, wk, bk, wv, bv, wo, bo,
        r1w, r1b, r2w, r2b, ln_g, ln_b,
    )
    nc = _get_program(L, LI, flags, [float(v) for v in np.asarray(r2b).ravel()])
    res = run_bass_kernel_spmd(nc, in_maps, core_ids=list(range(N_CORES)), trace=trace)
    out = np.empty((Bc, L, Hc), np.float32)
    for c in range(N_CORES):
        b, half = c // per_batch, c % per_batch
        out[b, half * LI : (half + 1) * LI] = res.results[c]["y"]
    if trace:
        return out, res
    return out
